# revision 43
# baseline (speedup 1.0000x reference)
"""LocallyConnected2d (non-overlapping 3x3 patches) Trainium2 kernel.

Problem: x [B=32, Cin=128, H=96, W=96], weight [Hout=32, Wout=32, Cout=128,
Cin=128, 3, 3], bias [Hout, Wout, Cout] -> out [B, Cout, Hout, Wout].

For each of the 1024 output positions (i, j) this is an independent
[B=32, K=1152] x [K=1152, Cout=128] matmul (K = Cin*KH*KW) plus bias.

Strategy:
  - Shard the 1024 positions over 8 NeuronCores by Hout rows (4 rows =
    128 positions per core).  The weight tensor dominates HBM traffic and
    position-sharding splits it evenly with zero duplication.
  - The kernel is HBM-bandwidth-bound (~360 GB/s per core fair share,
    ~320 achieved), so bytes are everything: weights AND x are quantized
    host-side to fp8 E3M4 (w: scale x128, x: scale x2.5, clip +-15.5;
    measured rel_max 0.0171 vs the 0.02 gate, bit-identical between HW
    and the host numpy simulation), halving both streams vs bf16; the
    output is stored bf16.  Host-side relayout makes every DMA
    descriptor a long contiguous run per SBUF partition:
        wk [kp=128, pos=128, ck=9, o=128]   (fp8e3, w*128)
        xk [kp=128, pos=128, ck=9, b=32]    (fp8e3, x*2.5)
    where the contraction index k = c*9 + p*3 + q is split as
    k = ck*128 + kp and kp sits on SBUF partitions.
  - Per position: 9 matmuls (lhsT = w chunk [128k x 128o] STATIONARY,
    fp8 + 128 cols triggers compiler-automatic Fast Weight Load;
    rhs = x chunk [128k x 32b] moving) accumulate into PSUM [128o, 32b].
  - Bias is NOT a matmul (a [1,128]-stationary bias matmul costs a full
    128-col LDWEIGHTS each -> ~17us of PE): instead a bias broadcast
    tile [o, pos*b] (values bias*scale, bf16) is DMA'd once at start and
    the PSUM->SBUF move becomes a DVE tensor_add(st, psum, bias_bc).
  - 16 positions share one PSUM bank [128, 512]; 64-position staging
    tiles are DMA'd to DRAM densely (output layout [o, pos, b], scaled
    down and transposed to [b, o, i, j] on host).
  - w loads ride nc.sync (HWDGE ring 0), x loads ride nc.scalar (HWDGE
    ring 1), output stores ride nc.gpsimd (SWDGE) so a blocked store
    never head-of-line blocks a prefetch.
  - Measured per-iteration decomposition (repeat=1001 differencing):
    full ~84us, input-DMA-only ~75us, PE+DVE-only ~44us -> DMA-bound
    with ~9us of MM/DMA SBUF contention; post-processing fully hidden.
"""

import numpy as np
import ml_dtypes

import concourse.bass as bass
import concourse.bacc as bacc
import concourse.mybir as mybir
import concourse.tile as tile
from concourse.bass_utils import run_bass_kernel_spmd

KH = KW = 3
B, CIN, H, W_IN = 32, 128, 96, 96
HOUT, WOUT, COUT = 32, 32, 128
NCORES = 8
IPC = HOUT // NCORES          # Hout rows per core = 4
POS = IPC * WOUT              # positions per core = 128
K = CIN * KH * KW             # 1152
CK = K // 128                 # 9 k-chunks of 128

WG = 8     # positions per weight-DMA tile
XG = 16    # positions per x-DMA tile
PG = 16    # positions per PSUM bank
SG = 64    # positions per output staging tile
WBUFS = 6  # weight pool buffers
XBUFS = 2  # x pool buffers
X_ON_ACT = True   # issue x DMAs on the scalar (ACT) HWDGE ring
X_FP8 = True      # quantize x to fp8 e3m4 as well (halves x DMA bytes)
W_FP8 = True      # quantize w to fp8 e3m4 (halves w DMA bytes)
BIAS_MM = False   # add bias via per-position matmul (old scheme) instead of DVE add
RING_SPLIT = False  # alternate w/x loads over both HWDGE rings, stores on SWDGE
OUT_ENG = "gpsimd"  # engine for output stores: "scalar" | "sync" | "gpsimd"
OUT_BF16 = True   # store output as bf16 (halves store bytes)
PPBUFS = 8        # PSUM pool buffers (8 banks max)
ALT_ADD = False   # alternate PSUM+bias adds between DVE and ACT
UNROLL = 8        # bodies per For_i trip in the repeat-loop bench
TILE_MAJOR = False  # DRAM layout tile-major: each DMA reads one contiguous block
WSCALE = 128.0    # w -> fp8 pre-scale (power of 2)
XSCALE = 2.5      # x -> fp8 pre-scale (only when X_FP8)

BF16 = mybir.dt.bfloat16
FP8 = mybir.dt.float8e3
FP32 = mybir.dt.float32
E3M4_MAX = 15.5

_NC_CACHE = {}


def set_config(**kw):
    g = globals()
    for k, v in kw.items():
        assert k in g, k
        g[k] = v
    _NC_CACHE.clear()


def _config_key():
    return (WG, XG, PG, SG, WBUFS, XBUFS, X_ON_ACT, X_FP8, W_FP8, BIAS_MM,
            RING_SPLIT, OUT_ENG, PPBUFS, ALT_ADD, OUT_BF16, UNROLL, TILE_MAJOR)


def _out_scale():
    return (WSCALE if W_FP8 else 1.0) * (XSCALE if X_FP8 else 1.0)


def _build_bass(repeat=1, variant="full"):
    """Build the Bass program. repeat>1 wraps the body in a dynamic loop
    (identical work each trip) so wall-clock timing can amortize the axon
    dispatch overhead: T(repeat) ~= overhead + repeat * T_kernel.
    variant: "full" | "dma" (input DMAs only) | "pe" (no input DMAs) |
    "empty" (loop overhead calibration)."""
    key = ("nc", repeat, variant, _config_key())
    if key in _NC_CACHE:
        return _NC_CACHE[key]
    xdt = FP8 if X_FP8 else BF16
    wdt = FP8 if W_FP8 else BF16
    nc = bacc.Bacc()
    if TILE_MAJOR:
        # Tile-major DRAM layouts: one DMA = one fully contiguous DRAM block
        # (partition runs adjacent), maximizing HBM row locality.
        xk = nc.declare_dram_parameter(
            "xk", [(POS // XG) * 128, XG * CK * B], xdt, isOutput=False)
        wk = nc.declare_dram_parameter(
            "wk", [(POS // WG) * 128, WG * CK * COUT], wdt, isOutput=False)
    else:
        xk = nc.declare_dram_parameter(
            "xk", [128, POS * CK * B], xdt, isOutput=False)
        wk = nc.declare_dram_parameter(
            "wk", [128, POS * CK * COUT], wdt, isOutput=False)
    bshape = [1, POS * COUT] if BIAS_MM else [COUT, POS * B]
    bk = nc.declare_dram_parameter("bk", bshape, BF16, isOutput=False)
    odt = BF16 if OUT_BF16 else FP32
    if TILE_MAJOR:
        out = nc.declare_dram_parameter(
            "out", [(POS // SG) * COUT, SG * B], odt, isOutput=True)
    else:
        out = nc.declare_dram_parameter(
            "out", [COUT, POS * B], odt, isOutput=True)

    with tile.TileContext(nc) as tc:
        with (
            tc.tile_pool(name="wpool", bufs=WBUFS) as wpool,
            tc.tile_pool(name="xpool", bufs=XBUFS) as xpool,
            tc.tile_pool(name="spool", bufs=2) as spool,
            tc.tile_pool(name="cpool", bufs=1) as cpool,
            tc.tile_pool(name="ppool", bufs=PPBUFS, space="PSUM") as ppool,
        ):
            bias_t = cpool.tile(bshape, BF16)
            nc.sync.dma_start(out=bias_t[:], in_=bk[:])
            ones = None
            if BIAS_MM:
                ones = cpool.tile([1, B], BF16)
                nc.vector.memset(ones[:], 1.0)

            def body():
                _emit_body(nc, tc, xk, wk, out, wpool, xpool, spool, ppool,
                           bias_t, ones, variant)

            if repeat == 1:
                body()
            else:
                # Unroll bodies per For_i trip: the loop boundary drains the
                # DMA prefetch pipeline (~5us bubble), so amortize it.
                unroll = next(
                    u for u in (UNROLL, 8, 4, 2, 1) if repeat % u == 0
                )
                with tc.For_i(0, repeat // unroll, 1):
                    for _ in range(unroll):
                        body()
    nc.finalize()
    _NC_CACHE[key] = nc
    return nc


def _emit_body(nc, tc, xk, wk, out, wpool, xpool, spool, ppool, bias_t, ones,
               variant="full"):
    xdt = FP8 if X_FP8 else BF16
    wdt = FP8 if W_FP8 else BF16
    odt = BF16 if OUT_BF16 else FP32
    XW = CK * B
    WW = CK * COUT
    use_dma = variant in ("full", "dma", "nopost", "nostore")
    use_pe = variant in ("full", "pe", "nopost", "nostore")
    use_post = variant not in ("nopost",)
    use_store = variant not in ("nopost", "nostore")
    if variant == "empty":
        nc.vector.memset(bias_t[0:1, 0:1], 0)
        return
    if variant == "dma":
        dummy = spool.tile([COUT, SG * B], odt, tag="dummy")
    wt = xt = st = pt = None
    for pos in range(POS):
        if pos % XG == 0:
            xt = xpool.tile([128, XG * XW], xdt)
            if use_dma:
                if RING_SPLIT:
                    xeng = nc.scalar if (pos // XG) % 2 == 0 else nc.sync
                else:
                    xeng = nc.scalar if X_ON_ACT else nc.sync
                if TILE_MAJOR:
                    xi = pos // XG
                    xsrc = xk[xi * 128 : (xi + 1) * 128, :]
                else:
                    xsrc = xk[:, pos * XW : (pos + XG) * XW]
                xeng.dma_start(out=xt[:], in_=xsrc)
            else:
                nc.vector.memset(xt[0:1, 0:1], 0)
            if not use_pe:
                nc.vector.tensor_copy(out=dummy[0:32, 0:64], in_=xt[0:32, 0:64])
        if pos % WG == 0:
            wt = wpool.tile([128, WG * WW], wdt)
            if use_dma:
                if RING_SPLIT:
                    weng = nc.sync if (pos // WG) % 2 == 0 else nc.scalar
                else:
                    weng = nc.sync
                if TILE_MAJOR:
                    wi = pos // WG
                    wsrc = wk[wi * 128 : (wi + 1) * 128, :]
                else:
                    wsrc = wk[:, pos * WW : (pos + WG) * WW]
                weng.dma_start(out=wt[:], in_=wsrc)
            else:
                nc.vector.memset(wt[0:1, 0:1], 0)
            if not use_pe:
                nc.vector.tensor_copy(out=dummy[0:32, 64:128], in_=wt[0:32, 0:64])
        if not use_pe:
            if pos == POS - 1:
                nc.scalar.dma_start(out=out[0:COUT, 0 : SG * B], in_=dummy[:])
            continue
        if pos % SG == 0:
            st = spool.tile([COUT, SG * B], odt)
        if pos % PG == 0:
            pt = ppool.tile([COUT, PG * B], FP32)

        xo = (pos % XG) * XW
        wo = (pos % WG) * WW
        po = (pos % PG) * B
        for ck in range(CK):
            nc.tensor.matmul(
                pt[:, po : po + B],
                wt[:, wo + ck * COUT : wo + (ck + 1) * COUT],
                xt[:, xo + ck * B : xo + (ck + 1) * B],
                start=(ck == 0),
                stop=(ck == CK - 1) and not BIAS_MM,
            )
        if BIAS_MM:
            nc.tensor.matmul(
                pt[:, po : po + B],
                bias_t[0:1, pos * COUT : (pos + 1) * COUT],
                ones[:],
                start=False,
                stop=True,
            )

        if use_post and pos % PG == PG - 1:
            p0 = pos - (PG - 1)
            so = (p0 % SG) * B
            veng = nc.vector
            if ALT_ADD and (pos // PG) % 2 == 1:
                veng = nc.scalar
            if BIAS_MM:
                veng.tensor_copy(out=st[:, so : so + PG * B], in_=pt[:])
            else:
                veng.tensor_add(
                    out=st[:, so : so + PG * B],
                    in0=pt[:],
                    in1=bias_t[:, p0 * B : p0 * B + PG * B],
                )
        if use_store and pos % SG == SG - 1:
            oeng = {"scalar": nc.scalar, "sync": nc.sync,
                    "gpsimd": nc.gpsimd}[OUT_ENG]
            if RING_SPLIT:
                oeng = nc.gpsimd
            if TILE_MAJOR:
                si = pos // SG
                odst = out[si * COUT : (si + 1) * COUT, :]
            else:
                q0 = (pos - (SG - 1)) * B
                odst = out[:, q0 : q0 + SG * B]
            oeng.dma_start(out=odst, in_=st[:])


def _prep_inputs(x, weight, bias):
    """Host-side cast + relayout. Returns per-core input maps."""
    if X_FP8:
        xq = np.clip(np.asarray(x, np.float32) * XSCALE, -E3M4_MAX, E3M4_MAX)
        xq = xq.astype(ml_dtypes.float8_e3m4)
    else:
        xq = np.asarray(x, dtype=np.float32).astype(ml_dtypes.bfloat16)
    if W_FP8:
        wq = np.clip(np.asarray(weight, np.float32) * WSCALE, -E3M4_MAX, E3M4_MAX)
        wq = wq.astype(ml_dtypes.float8_e3m4)
    else:
        wq = np.asarray(weight, dtype=np.float32).astype(ml_dtypes.bfloat16)
    bb = np.asarray(bias, dtype=np.float32) * _out_scale()

    # x: [b, c, i, p, j, q] -> [i, j, k=(c,p,q), b] -> split k -> [i,j,ck,kp,b]
    xt = (
        xq.reshape(B, CIN, HOUT, KH, WOUT, KW)
        .transpose(2, 4, 1, 3, 5, 0)
        .reshape(HOUT, WOUT, K, B)
        .reshape(HOUT, WOUT, CK, 128, B)
    )
    # w: [i, j, o, c, p, q] -> [i, j, k, o] -> [i, j, ck, kp, o]
    wt = (
        wq.transpose(0, 1, 3, 4, 5, 2)
        .reshape(HOUT, WOUT, K, COUT)
        .reshape(HOUT, WOUT, CK, 128, COUT)
    )

    in_maps = []
    for c in range(NCORES):
        i0 = c * IPC
        # -> [kp, il, j, ck, {b|o}] so each SBUF partition (kp) reads one
        # long contiguous DRAM run per DMA.
        xc = np.ascontiguousarray(
            xt[i0 : i0 + IPC].transpose(3, 0, 1, 2, 4)
        ).reshape(128, POS * CK * B)
        wc = np.ascontiguousarray(
            wt[i0 : i0 + IPC].transpose(3, 0, 1, 2, 4)
        ).reshape(128, POS * CK * COUT)
        if TILE_MAJOR:
            # [kp, nt, tilecols] -> [nt, kp, tilecols]: each DMA tile becomes
            # one contiguous DRAM block.
            ntx, ntw = POS // XG, POS // WG
            xc = np.ascontiguousarray(
                xc.reshape(128, ntx, XG * CK * B).transpose(1, 0, 2)
            ).reshape(ntx * 128, XG * CK * B)
            wc = np.ascontiguousarray(
                wc.reshape(128, ntw, WG * CK * COUT).transpose(1, 0, 2)
            ).reshape(ntw * 128, WG * CK * COUT)
        if BIAS_MM:
            bc = np.ascontiguousarray(bb[i0 : i0 + IPC]).reshape(
                1, POS * COUT
            ).astype(ml_dtypes.bfloat16)
        else:
            # bias broadcast [o, pos*b]: col = pos*B + b, value bias[i,j,o]*scale
            bc = np.ascontiguousarray(
                np.broadcast_to(
                    bb[i0 : i0 + IPC].transpose(2, 0, 1)[:, :, :, None],
                    (COUT, IPC, WOUT, B),
                )
            ).reshape(COUT, POS * B).astype(ml_dtypes.bfloat16)
        in_maps.append({"xk": xc, "wk": wc, "bk": bc})
    return in_maps


def _assemble(results):
    out = np.empty((B, COUT, HOUT, WOUT), dtype=np.float32)
    inv = 1.0 / _out_scale()
    for c in range(NCORES):
        r = np.asarray(results[c]["out"], dtype=np.float32)
        if TILE_MAJOR:
            # [nt*o, SG*b] -> [o, pos*b]
            nts = POS // SG
            r = r.reshape(nts, COUT, SG * B).transpose(1, 0, 2).reshape(
                COUT, POS * B
            )
        # [o, pos*b] -> [o, il, j, b] -> [b, o, il, j]
        out[:, :, c * IPC : (c + 1) * IPC, :] = (
            r.reshape(COUT, IPC, WOUT, B).transpose(3, 0, 1, 2) * inv
        )
    return out


def _run(inputs, trace=False, **kw):
    in_maps = _prep_inputs(inputs["x"], inputs["weight"], inputs["bias"])
    nc = _build_bass()
    res = run_bass_kernel_spmd(nc, in_maps, list(range(NCORES)), trace=trace, **kw)
    return _assemble(res.results), res


def kernel(**inputs) -> np.ndarray:
    out, _ = _run(inputs, trace=False)
    return out


def _make_exec(nc, in_maps):
    """Build the sharded jitted executable for nc and device-resident args.
    Returns (fn, dev_args)."""
    import jax
    from jax.sharding import Mesh, PartitionSpec
    from jax.experimental.shard_map import shard_map
    from concourse import bass2jax, mybir as mb

    bass2jax.install_neuronx_cc_hook()

    partition_name = (
        nc.partition_id_tensor.name if nc.partition_id_tensor else None
    )
    in_names, out_names, out_avals, zero_outs = [], [], [], []
    for alloc in nc.m.functions[0].allocations:
        if not isinstance(alloc, mb.MemoryLocationSet):
            continue
        name = alloc.memorylocations[0].name
        if alloc.kind == "ExternalInput":
            if name != partition_name:
                in_names.append(name)
        elif alloc.kind == "ExternalOutput":
            out_names.append(name)
            shape = tuple(alloc.tensor_shape)
            dtype = mb.dt.np(alloc.dtype)
            out_avals.append(jax.core.ShapedArray(shape, dtype))
            zero_outs.append(np.zeros(shape, dtype))
    n_params = len(in_names)
    all_in_names = in_names + out_names
    if partition_name is not None:
        all_in_names = all_in_names + [partition_name]

    def _body(*args):
        operands = list(args)
        if partition_name is not None:
            operands.append(bass2jax.partition_id_tensor())
        outs = bass2jax._bass_exec_p.bind(
            *operands,
            out_avals=tuple(out_avals),
            in_names=tuple(all_in_names),
            out_names=tuple(out_names),
            lowering_input_output_aliases=(),
            sim_require_finite=True,
            sim_require_nnan=True,
            nc=nc,
        )
        return tuple(outs)

    devices = jax.devices()[:NCORES]
    mesh = Mesh(np.asarray(devices), ("core",))
    n_outs = len(out_names)
    fn = jax.jit(
        shard_map(
            _body,
            mesh=mesh,
            in_specs=(PartitionSpec("core"),) * (n_params + n_outs),
            out_specs=(PartitionSpec("core"),) * n_outs,
            check_rep=False,
        ),
        keep_unused=True,
    )
    concat_in = [
        np.concatenate([np.asarray(m[name]) for m in in_maps], axis=0)
        for name in in_names
    ]
    concat_zeros = [
        np.zeros((NCORES * z.shape[0], *z.shape[1:]), z.dtype) for z in zero_outs
    ]
    sharding = jax.sharding.NamedSharding(mesh, PartitionSpec("core"))
    dev_in = [jax.device_put(a, sharding) for a in concat_in]
    dev_zeros = [jax.device_put(a, sharding) for a in concat_zeros]
    return fn, dev_in + dev_zeros


def _timed_exec(nc, in_maps, n_iters):
    """Compile nc via the bass2jax path, keep inputs device-resident, and
    return the min wall-clock seconds over n_iters calls."""
    import time

    import jax

    fn, dev_args = _make_exec(nc, in_maps)
    # warmup (compiles)
    r = fn(*dev_args)
    jax.block_until_ready(r)
    times = []
    for _ in range(n_iters):
        t0 = time.perf_counter()
        r = fn(*dev_args)
        jax.block_until_ready(r)
        times.append(time.perf_counter() - t0)
    print(f"    raw times (ms): {[f'{t * 1e3:.2f}' for t in times]}")
    # median: the axon dispatch constant is bimodal (~60ms rare / ~100ms
    # typical), so min() is a trap; medians are tight (+-0.5ms).
    return float(np.median(times)), r


def bench(inputs, r_small=1, r_big=1000, n_iters=9):
    """Estimate per-kernel HW time by differencing two repeat counts.
    r_big=1001 gives ~60-90ms of kernel time on top of the ~90ms axon
    dispatch constant, so the slope is well above the +-2-5ms dispatch
    jitter (r_big=41 was measurably noise-dominated)."""
    in_maps = _prep_inputs(inputs["x"], inputs["weight"], inputs["bias"])
    t_small, _ = _timed_exec(_build_bass(repeat=r_small), in_maps, n_iters)
    t_big, _ = _timed_exec(_build_bass(repeat=r_big), in_maps, n_iters)
    ns = (t_big - t_small) / (r_big - r_small) * 1e9
    print(
        f"bench: T({r_small})={t_small * 1e3:.3f} ms  T({r_big})={t_big * 1e3:.3f} ms"
        f"  -> per-kernel {ns:.0f} ns"
    )
    return ns
